# revision 23
# baseline (speedup 1.0000x reference)
"""Trainium2 Bass kernel for a dense transformer block (causal MHA + FFN, post-LN).

Sharding over 8 NeuronCores:
  - Attention is tensor-parallel over heads: core c computes heads 2c, 2c+1
    for all 4096 tokens (B*T flattened, batch-major).
  - One AllToAll per batch redistributes the per-head attention outputs so
    core c ends up with the full head-concatenated attention output
    (transposed) for its token half-slices.
  - Wo + residual + LN1 + FFN + residual + LN2 are sequence-parallel: each
    core processes its 2x256 token rows and outputs [512, 1024].

Precision/scaling (rel-err gate 2e-2, this config measures ~1.4e-2):
  - QKV / Wo / FFN1 / FFN2 matmuls in fp8e4m3 with DoubleRow (2 k-tiles per
    pass, measured 2x vs bf16 at N=512). Weights pre-scaled x16/x32 host-side
    to clear fp8 subnormals; the scales ride in the activations and wash out
    in the LayerNorms (eps scaled to match).
  - Attention internals bf16; softmax denominator via a ones-column in P@V;
    exp restricted to the causally-valid range of diagonal tiles; the score
    stream runs one j ahead of the exp stream so ACT never starves.
"""

import sys

sys.path.insert(0, "/opt/trn_rl_repo")

import numpy as np
import ml_dtypes

B, T, E, H = 2, 2048, 1024, 16
HS = E // H  # 64
N_CORES = 8
HPC = H // N_CORES  # heads per core = 2
NTOK = B * T  # 4096
TSL = NTOK // N_CORES  # 512 token rows per core
HSL = TSL // B  # 256 rows per (core, batch) half-slice
EPS = 1e-5

BF16 = ml_dtypes.bfloat16
FP8 = ml_dtypes.float8_e4m3
EO_ = E // 128  # 8
FO_ = 4 * E // 128  # 32

SW = 16.0          # fp8 weight pre-scale (Wq/Wk/Wv/Wo/W1)
S_LN1_IN = 256.0   # wo psum scale: (16 att)(16 Wo)
S_LN1_OUT = 32.0   # LN1 output scale (host-scaled g1, be1)
S_FF = 32.0        # W2 pre-scale; ffn2 psum/residual scale

_cache = {}


def _build(n_cores=N_CORES):
    import concourse.bass as bass
    import concourse.tile as tile
    import concourse.bacc as bacc
    from concourse import mybir

    BF = mybir.dt.bfloat16
    F32 = mybir.dt.float32
    F8 = mybir.dt.float8e4
    AF = mybir.ActivationFunctionType
    OP = mybir.AluOpType
    DR = mybir.MatmulPerfMode.DoubleRow

    nc = bacc.Bacc("TRN2", target_bir_lowering=False, debug=False,
                   num_devices=n_cores)

    EO = E // 128            # 8 chunks of the embedding dim
    FO = 4 * E // 128        # 32 chunks of the FFN hidden dim
    TC = T // 512            # 4 t-chunks of 512 per batch

    xT_d = nc.dram_tensor("xT", [128, NTOK // 512, EO, 512], F8,
                          kind="ExternalInput")
    xsl_d = nc.dram_tensor("x_slice", [128, TSL // 128, E], F32,
                           kind="ExternalInput")
    wq_d = nc.dram_tensor("wq", [128, EO, HPC * HS], F8, kind="ExternalInput")
    wk_d = nc.dram_tensor("wk", [128, EO, HPC * HS], F8, kind="ExternalInput")
    wv_d = nc.dram_tensor("wv", [128, EO, HPC * HS], F8, kind="ExternalInput")
    wo_d = nc.dram_tensor("wo", [128, EO, E], F8, kind="ExternalInput")
    w1_d = nc.dram_tensor("w1", [128, FO, EO, 128], F8, kind="ExternalInput")
    w2_d = nc.dram_tensor("w2", [128, FO, E], F8, kind="ExternalInput")
    b1s_d = nc.dram_tensor("b1s", [128, FO], F32, kind="ExternalInput")
    bo_d = nc.dram_tensor("bo", [E], F32, kind="ExternalInput")     # 256*bo
    b2r_d = nc.dram_tensor("b2r", [1, E], BF, kind="ExternalInput")  # 32*b2
    g1_d = nc.dram_tensor("g1", [E], BF, kind="ExternalInput")      # 32*g1
    be1_d = nc.dram_tensor("be1", [E], BF, kind="ExternalInput")    # 32*be1
    g2_d = nc.dram_tensor("g2", [E], BF, kind="ExternalInput")
    be2_d = nc.dram_tensor("be2", [E], BF, kind="ExternalInput")
    masks_d = nc.dram_tensor("masks", [128, 128], BF, kind="ExternalInput")
    idb_d = nc.dram_tensor("id_bf", [128, 128], BF, kind="ExternalInput")
    out_d = nc.dram_tensor("out", [TSL, E], BF, kind="ExternalOutput")

    def bcast_ap(d, n):
        a = d.ap()
        return bass.AP(tensor=a.tensor, offset=a.offset, ap=[[0, 128], [1, n]])

    with tile.TileContext(nc) as tc:
        with tc.tile_pool(name="dram", bufs=1, space="DRAM") as dram, \
             tc.tile_pool(name="consts", bufs=1) as consts:

            a2a_in = [dram.tile([n_cores, 128, HSL], BF, name=f"a2a_in{b}")
                      for b in range(B)]
            a2a_out = [dram.tile([n_cores, 128, HSL], BF, name=f"a2a_out{b}")
                       for b in range(B)]

            # ---- attention-critical constants first -----------------------
            wq_sb = consts.tile([128, EO, HPC * HS], F8)
            nc.sync.dma_start(wq_sb[:], wq_d.ap())
            wk_sb = consts.tile([128, EO, HPC * HS], F8)
            wv_sb = consts.tile([128, EO, HPC * HS], F8)
            masks_sb = consts.tile([128, 128], BF)
            nc.scalar.dma_start(masks_sb[:], masks_d.ap())
            idb_sb = consts.tile([128, 128], BF)
            nc.scalar.dma_start(idb_sb[:], idb_d.ap())
            eps1_sb = consts.tile([128, 1], F32)
            nc.vector.memset(eps1_sb[:], S_LN1_IN * S_LN1_IN * EPS)
            eps2_sb = consts.tile([128, 1], F32)
            nc.vector.memset(eps2_sb[:], S_FF * S_FF * EPS)
            # back-half constants: issued up-front on the (idle) gpsimd
            # DMA queue so they stream in during attention
            b1_sb = consts.tile([128, FO], F32)
            nc.gpsimd.dma_start(b1_sb[:], b1s_d.ap())
            bo_bc = consts.tile([128, E], F32)
            nc.gpsimd.dma_start(bo_bc[:], bcast_ap(bo_d, E))
            b2r_sb = consts.tile([1, E], BF)
            nc.gpsimd.dma_start(b2r_sb[:], b2r_d.ap())
            ones_row = consts.tile([1, 128], BF)
            nc.vector.memset(ones_row[:], 1.0)
            g1_bc = consts.tile([128, E], BF)
            nc.gpsimd.dma_start(g1_bc[:], bcast_ap(g1_d, E))
            be1_bc = consts.tile([128, E], BF)
            nc.gpsimd.dma_start(be1_bc[:], bcast_ap(be1_d, E))
            g2_bc = consts.tile([128, E], BF)
            nc.gpsimd.dma_start(g2_bc[:], bcast_ap(g2_d, E))
            be2_bc = consts.tile([128, E], BF)
            nc.gpsimd.dma_start(be2_bc[:], bcast_ap(be2_d, E))

            hcT_sb = [consts.tile([128, EO, HSL], BF, name=f"hcT{h2}")
                      for h2 in range(B)]
            hc8_sb = [consts.tile([128, EO, HSL], F8, name=f"hc8_{h2}")
                      for h2 in range(B)]
            wo_sb = consts.tile([128, EO, E], F8)
            w1_sb = consts.tile([128, FO, EO, 128], F8)
            w2_sb = consts.tile([128, FO, E], F8)
            xpb_sb = consts.tile([128, TSL // 128, E], F32)  # -> 256*(x+bo)

            # ================= attention (heads 2c, 2c+1) =================
            with tc.tile_pool(name="att_big", bufs=1) as att_big, \
                 tc.tile_pool(name="att_qkv", bufs=2) as att_qkv, \
                 tc.tile_pool(name="att_pt", bufs=4) as att_pt, \
                 tc.tile_pool(name="att_small", bufs=4) as att_small, \
                 tc.tile_pool(name="ps_big", bufs=2, space="PSUM") as ps_big, \
                 tc.tile_pool(name="ps_small", bufs=2, space="PSUM") as ps_small, \
                 tc.tile_pool(name="ps_av", bufs=2, space="PSUM") as ps_av:
                ps_qk = ps_s = ps_big          # share 2x 2-bank slots (tag "qs")
                ps_v = ps_tp = ps_small        # share 2x 1-bank slots (tag "vtp")

                xT_sb = att_big.tile([128, NTOK // 512, EO, 512], F8, tag="xT")
                nc.sync.dma_start(xT_sb[:, 0], xT_d.ap()[:, 0])
                nc.sync.dma_start(wk_sb[:], wk_d.ap())
                nc.sync.dma_start(wv_sb[:], wv_d.ap())
                for sl_i in range(1, NTOK // 512):
                    nc.sync.dma_start(xT_sb[:, sl_i], xT_d.ap()[:, sl_i])
                # prefetch the whole back half behind the attention inputs
                # (needed-first order: wo/xpb ~150us, w1 ~190, w2 ~230)
                nc.sync.dma_start(wo_sb[:], wo_d.ap())
                nc.sync.dma_start(xpb_sb[:], xsl_d.ap())
                nc.sync.dma_start(w1_sb[:], w1_d.ap())
                nc.sync.dma_start(w2_sb[:], w2_d.ap())

                def alloc_qkv(b):
                    return (att_qkv.tile([128, T], BF, tag="qT",
                                         name=f"qT{b}"),
                            att_qkv.tile([128, T], BF, tag="kT",
                                         name=f"kT{b}"),
                            att_qkv.tile([128, T // 128, 2 * (HS + 1)], BF,
                                         tag="v", name=f"v{b}"))

                def emit_qk_chunk(b, ci, qT_sb, kT_sb):
                    cg = b * TC + ci
                    qk_ps = ps_qk.tile([128, 2, 512], F32, tag="qs",
                                       name=f"qk{b}_{ci}")
                    for ep in range(EO // 2):
                        nc.tensor.matmul(
                            qk_ps[:, 0, :], wq_sb[:, 2 * ep:2 * ep + 2, :],
                            xT_sb[:, cg, 2 * ep:2 * ep + 2, :],
                            start=ep == 0, stop=ep == EO // 2 - 1,
                            perf_mode=DR)
                    for ep in range(EO // 2):
                        nc.tensor.matmul(
                            qk_ps[:, 1, :], wk_sb[:, 2 * ep:2 * ep + 2, :],
                            xT_sb[:, cg, 2 * ep:2 * ep + 2, :],
                            start=ep == 0, stop=ep == EO // 2 - 1,
                            perf_mode=DR)
                    nc.vector.tensor_copy(
                        qT_sb[:, 512 * ci:512 * ci + 512], qk_ps[:, 0, :])
                    nc.vector.tensor_copy(
                        kT_sb[:, 512 * ci:512 * ci + 512], qk_ps[:, 1, :])

                def emit_v_unit(b, ci, k2, v_sb):
                    cg = b * TC + ci
                    vp = ps_v.tile([128, 128], F32, tag="vtp",
                                   name=f"vp{b}_{ci}_{k2}")
                    for ep in range(EO // 2):
                        nc.tensor.matmul(
                            vp[:],
                            xT_sb[:, cg, 2 * ep:2 * ep + 2,
                                  128 * k2:128 * (k2 + 1)],
                            wv_sb[:, 2 * ep:2 * ep + 2, :],
                            start=ep == 0, stop=ep == EO // 2 - 1,
                            perf_mode=DR)
                    ts_ = 4 * ci + k2
                    vrow = v_sb[:, ts_, :]
                    # ones columns at 64 and 129
                    ones_view = bass.AP(
                        tensor=vrow.tensor, offset=vrow.offset + HS,
                        ap=[vrow.ap[0], [HS + 1, 2]])
                    nc.vector.memset(ones_view, 1.0)
                    dst = bass.AP(
                        tensor=vrow.tensor, offset=vrow.offset,
                        ap=[vrow.ap[0], [HS + 1, 2], [1, HS]])
                    nc.vector.tensor_copy(
                        dst, vp[:].rearrange("p (h d) -> p h d", h=2))

                def emit_attn(b, qT_sb, kT_sb, v_sb, filler_v=None):
                    # filler_v: v_sb of the NEXT batch; its v-units are paced
                    # into the j-loop to fill the PE's exp-wait gaps
                    for i in range(TC):
                        av_ps = [ps_av.tile([128, 2, 2, HS + 1], F32,
                                            tag="av", name=f"av_{i}_{p}")
                                 for p in range(2)]
                        nj = 4 * i + 4
                        s_tiles = {}
                        vdone = 0

                        def emit_scores(j, i=i, s_tiles=s_tiles):
                            s_ps = ps_s.tile([128, 2, 512], F32, tag="qs",
                                             name=f"s{b}_{i}_{j}")
                            for h in range(2):
                                nc.tensor.matmul(
                                    s_ps[:, h, :],
                                    kT_sb[64 * h:64 * h + 64,
                                          128 * j:128 * j + 128],
                                    qT_sb[64 * h:64 * h + 64,
                                          512 * i:512 * i + 512],
                                    start=True, stop=True)
                            s_tiles[j] = s_ps

                        emit_scores(0)
                        for j in range(nj):
                            # scores one j ahead: the exp stream on ACT
                            # never waits for score production
                            if j + 1 < nj:
                                emit_scores(j + 1)
                            q = j - 4 * i
                            t0_ = 128 * q if q > 0 else 0
                            s_ps = s_tiles.pop(j)
                            pt = att_pt.tile([128, 2, 512], BF, tag="pt")
                            nc.scalar.activation(
                                pt[:, :, t0_:], s_ps[:, :, t0_:], AF.Exp,
                                scale=1.0 / (np.sqrt(HS) * SW * SW))
                            if q >= 0:
                                nc.vector.tensor_tensor(
                                    pt[:, :, 128 * q:128 * (q + 1)],
                                    pt[:, :, 128 * q:128 * (q + 1)],
                                    masks_sb[:, None, :].to_broadcast(
                                        (128, 2, 128)),
                                    OP.mult)
                            for k2 in range(4):
                                if j > 4 * i + k2:
                                    continue
                                for h in range(2):
                                    # start=True clears has_written for the
                                    # WHOLE bank, so only the very first
                                    # matmul into each bank may set it.
                                    nc.tensor.matmul(
                                        av_ps[k2 // 2][:, k2 % 2, h, :],
                                        pt[:, h, 128 * k2:128 * (k2 + 1)],
                                        v_sb[:, j, (HS + 1) * h:(HS + 1) * (h + 1)],
                                        start=(j == 0 and h == 0
                                               and k2 % 2 == 0),
                                        stop=j == 4 * i + k2)
                            if filler_v is not None:
                                want = 4 * (j + 1) // nj
                                while vdone < want:
                                    emit_v_unit(1, i, vdone, filler_v)
                                    vdone += 1
                        for k2 in range(4):
                            avp = av_ps[k2 // 2][:, k2 % 2, :, :]
                            recip = att_small.tile([128, 2], F32, tag="recip")
                            nc.vector.reciprocal(recip[:], avp[:, :, HS])
                            onorm = att_small.tile([128, 128], BF, tag="onorm")
                            for h in range(2):
                                nc.vector.tensor_scalar_mul(
                                    onorm[:, 64 * h:64 * h + 64],
                                    avp[:, h, 0:HS],
                                    recip[:, h:h + 1])
                            tp = ps_tp.tile([128, 128], BF, tag="vtp")
                            nc.tensor.transpose(tp[:], onorm[:], idb_sb[:])
                            ot = att_small.tile([128, 128], BF, tag="ot")
                            nc.vector.tensor_copy(ot[:], tp[:])
                            g2_ = 512 * i + 128 * k2  # within-batch col
                            nc.scalar.dma_start(
                                a2a_in[b][g2_ // HSL, :,
                                          (g2_ % HSL):(g2_ % HSL) + 128],
                                ot[:])

                with nc.named_scope("qkv0"):
                    qT0, kT0, v0 = alloc_qkv(0)
                    for ci in range(TC):
                        emit_qk_chunk(0, ci, qT0, kT0)
                        for k2 in range(4):
                            emit_v_unit(0, ci, k2, v0)

                qT1, kT1, v1 = alloc_qkv(1)
                with nc.named_scope("attn0"):
                    emit_attn(0, qT0, kT0, v0, filler_v=v1)

                # xpb = 256*x + 256*bo, emitted here so the DVE does it in
                # the attn1 shadow (at t0 these ops would stall qkv0's
                # copies behind the prefetch DMAs)
                for m_ in range(TSL // 128):
                    nc.vector.tensor_scalar_mul(
                        xpb_sb[:, m_, :], xpb_sb[:, m_, :], S_LN1_IN)
                    nc.vector.tensor_tensor(
                        xpb_sb[:, m_, :], xpb_sb[:, m_, :],
                        bo_bc[:], OP.add)

                with nc.named_scope("a2a0"):
                    nc.gpsimd.collective_compute(
                        "AllToAll", mybir.AluOpType.bypass,
                        replica_groups=[list(range(n_cores))],
                        ins=[a2a_in[0].opt()], outs=[a2a_out[0].opt()])
                nc.sync.dma_start(
                    hcT_sb[0][:], a2a_out[0][:].rearrange("i p t -> p i t"))
                nc.gpsimd.tensor_copy(hc8_sb[0][:], hcT_sb[0][:])

                with nc.named_scope("qkv1"):
                    for ci in range(TC):
                        emit_qk_chunk(1, ci, qT1, kT1)

                with nc.named_scope("attn1"):
                    emit_attn(1, qT1, kT1, v1)

                with nc.named_scope("a2a1"):
                    nc.gpsimd.collective_compute(
                        "AllToAll", mybir.AluOpType.bypass,
                        replica_groups=[list(range(n_cores))],
                        ins=[a2a_in[1].opt()], outs=[a2a_out[1].opt()])
                nc.sync.dma_start(
                    hcT_sb[1][:], a2a_out[1][:].rearrange("i p t -> p i t"))
                nc.gpsimd.tensor_copy(hc8_sb[1][:], hcT_sb[1][:])

            # ================= back half (2 x 256-token half-slices) =====
            with tc.tile_pool(name="bh", bufs=1) as bh, \
                 tc.tile_pool(name="bh_w", bufs=2) as bh_w, \
                 tc.tile_pool(name="bh_small", bufs=4) as bh_small:

                x1f = bh.tile([128, TSL // 128, E], BF, tag="x1f")
                x1b = bh.tile([128, TSL // 128, E], F32, tag="x1b")
                x1T = bh.tile([128, EO, TSL], F8, tag="x1T")
                hT = bh.tile([128, FO, TSL], F8, tag="hT")
                out_bf = bh.tile([128, TSL // 128, E], BF, tag="out_bf")
                out_dst = out_d.ap().rearrange("(m p) e -> p m e", p=128)

                def layernorm(buf_m, g_bc, be_bc, eps_sb, out_m=None):
                    stats = bh_small.tile([128, 2, 6], F32, tag="stats")
                    for s2 in range(2):
                        nc.vector.bn_stats(stats[:, s2, :],
                                           buf_m[:, 512 * s2:512 * (s2 + 1)])
                    mv = bh_small.tile([128, 2], F32, tag="mv")
                    nc.vector.bn_aggr(mv[:], stats[:])
                    std = bh_small.tile([128, 1], F32, tag="std")
                    nc.scalar.activation(std[:], mv[:, 1:2], AF.Sqrt,
                                         bias=eps_sb[:, 0:1])
                    rstd = bh_small.tile([128, 1], F32, tag="rstd")
                    nc.vector.reciprocal(rstd[:], std[:])
                    nc.vector.tensor_scalar(
                        buf_m[:], buf_m[:], mv[:, 0:1], rstd[:],
                        op0=OP.subtract, op1=OP.mult)
                    nc.vector.tensor_tensor(buf_m[:], buf_m[:], g_bc[:], OP.mult)
                    nc.vector.tensor_tensor(out_m if out_m is not None
                                            else buf_m[:],
                                            buf_m[:], be_bc[:], OP.add)

                M2N = HSL // 128  # 2 row-subtiles per half
                for h2 in range(B):
                    ms = [M2N * h2 + m2 for m2 in range(M2N)]
                    tsl = slice(256 * h2, 256 * (h2 + 1))
                    with nc.named_scope(f"bh_h{h2}"):
                        with tc.tile_pool(name=f"ps_wo{h2}", bufs=2,
                                          space="PSUM") as ps_wo, \
                             tc.tile_pool(name=f"ps_tp2_{h2}", bufs=4,
                                          space="PSUM") as ps_tp2:
                            hc8 = hc8_sb[h2]
                            for m2 in range(M2N):
                                m = ms[m2]
                                for n in range(2):
                                    wo_ps = ps_wo.tile([128, 512], F32,
                                                       tag="wo")
                                    for hp in range(EO // 2):
                                        nc.tensor.matmul(
                                            wo_ps[:],
                                            hc8[:, 2 * hp:2 * hp + 2,
                                                128 * m2:128 * (m2 + 1)],
                                            wo_sb[:, 2 * hp:2 * hp + 2,
                                                  512 * n:512 * (n + 1)],
                                            start=hp == 0,
                                            stop=hp == EO // 2 - 1,
                                            perf_mode=DR)
                                    sl = slice(512 * n, 512 * (n + 1))
                                    nc.vector.tensor_tensor(
                                        x1f[:, m, sl], wo_ps[:],
                                        xpb_sb[:, m, sl], OP.add)
                            for m2 in range(M2N):
                                # LN(m) then its transposes immediately: the
                                # PE's tp work for m overlaps LN(m+1) on DVE
                                m = ms[m2]
                                layernorm(x1f[:, m, :], g1_bc, be1_bc,
                                          eps1_sb)
                                for eo in range(EO):
                                    tp2 = ps_tp2.tile([128, 128], BF,
                                                      tag="tp2")
                                    nc.tensor.transpose(
                                        tp2[:],
                                        x1f[:, m, 128 * eo:128 * (eo + 1)],
                                        idb_sb[:])
                                    nc.vector.tensor_copy(
                                        x1T[:, eo, 128 * m:128 * (m + 1)],
                                        tp2[:])

                        # ffn for this half: ffn1 (N=256) pipelined with the
                        # DoubleRow ffn2 on fo pairs; h0's ffn runs inside the
                        # a2a1 peer-skew window
                        with tc.tile_pool(name=f"ps_f1_{h2}", bufs=3,
                                          space="PSUM") as ps_f1, \
                             tc.tile_pool(name=f"ps_f2_{h2}", bufs=1,
                                          space="PSUM") as ps_f2:
                            f2_ps = {}
                            for m in ms:
                                for n in range(2):
                                    f2_ps[m, n] = ps_f2.tile(
                                        [128, 512], F32, tag=f"f2_{m % 2}_{n}",
                                        name=f"f2_{m}_{n}")
                                    # K=1 bf16 matmul seeds the bank with 32*b2
                                    nc.tensor.matmul(
                                        f2_ps[m, n][:], ones_row[:],
                                        b2r_sb[:, 512 * n:512 * (n + 1)],
                                        start=True, stop=False,
                                        skip_group_check=True)
                            for fo in range(FO):
                                f1_ps = ps_f1.tile([128, 256], F32, tag="f1")
                                for ep in range(EO // 2):
                                    nc.tensor.matmul(
                                        f1_ps[:],
                                        w1_sb[:, fo, 2 * ep:2 * ep + 2, :],
                                        x1T[:, 2 * ep:2 * ep + 2, tsl],
                                        start=ep == 0, stop=ep == EO // 2 - 1,
                                        perf_mode=DR)
                                nc.scalar.activation(
                                    hT[:, fo, tsl], f1_ps[:], AF.Relu,
                                    bias=b1_sb[:, fo:fo + 1],
                                    scale=1.0 / (S_LN1_OUT * SW))
                                if fo % 2 == 1:
                                    for m in ms:
                                        for n in range(2):
                                            nc.tensor.matmul(
                                                f2_ps[m, n][:],
                                                hT[:, fo - 1:fo + 1,
                                                   128 * m:128 * (m + 1)],
                                                w2_sb[:, fo - 1:fo + 1,
                                                      512 * n:512 * (n + 1)],
                                                start=False,
                                                stop=fo == FO - 1,
                                                perf_mode=DR)
                            for m in ms:
                                for n in range(2):
                                    sl = slice(512 * n, 512 * (n + 1))
                                    nc.vector.tensor_tensor(
                                        x1b[:, m, sl], f2_ps[m, n][:],
                                        x1f[:, m, sl], OP.add)
                                layernorm(x1b[:, m, :], g2_bc, be2_bc,
                                          eps2_sb, out_m=out_bf[:, m, :])
                                nc.sync.dma_start(out_dst[:, m, :],
                                                  out_bf[:, m, :])

    nc.compile()
    return nc


def _make_in_maps(inputs):
    x = np.asarray(inputs["x"], dtype=np.float32)
    Wq = np.asarray(inputs["Wq"], dtype=np.float32)
    Wk = np.asarray(inputs["Wk"], dtype=np.float32)
    Wv = np.asarray(inputs["Wv"], dtype=np.float32)
    Wo = np.asarray(inputs["Wo"], dtype=np.float32)

    xflat = x.reshape(NTOK, E)
    xT = np.ascontiguousarray(
        xflat.reshape(NTOK // 512, 512, EO_, 128).transpose(3, 0, 2, 1)
    ).astype(FP8)
    wo = np.ascontiguousarray(
        (SW * Wo).reshape(EO_, 128, E).transpose(1, 0, 2)).astype(FP8)
    w1 = np.ascontiguousarray(
        (SW * np.asarray(inputs["W1"], dtype=np.float32))
        .reshape(EO_, 128, FO_, 128).transpose(1, 2, 0, 3)
    ).astype(FP8)
    w2 = np.ascontiguousarray(
        (S_FF * np.asarray(inputs["W2"], dtype=np.float32))
        .reshape(FO_, 128, E).transpose(1, 0, 2)).astype(FP8)
    b1s = np.ascontiguousarray(
        np.asarray(inputs["b1"], dtype=np.float32).reshape(FO_, 128).T)

    srow = np.arange(128)[:, None]
    tcol = np.arange(128)[None, :]
    masks = np.ascontiguousarray((srow <= tcol).astype(np.float32)).astype(BF16)

    ident = np.eye(128, dtype=np.float32)

    common = {
        "xT": xT,
        "wo": wo,
        "w1": w1,
        "w2": w2,
        "b1s": b1s,
        "bo": S_LN1_IN * np.asarray(inputs["bo"], dtype=np.float32),
        "b2r": (S_FF * np.asarray(inputs["b2"], dtype=np.float32)
                ).astype(BF16).reshape(1, E),
        "g1": (S_LN1_OUT * np.asarray(inputs["g1"], dtype=np.float32)
               ).astype(BF16),
        "be1": (S_LN1_OUT * np.asarray(inputs["be1"], dtype=np.float32)
                ).astype(BF16),
        "g2": np.asarray(inputs["g2"], dtype=np.float32).astype(BF16),
        "be2": np.asarray(inputs["be2"], dtype=np.float32).astype(BF16),
        "masks": masks,
        "id_bf": ident.astype(BF16),
    }
    in_maps = []
    for c in range(N_CORES):
        m = dict(common)
        def tile_w(W):
            wc = np.concatenate([W[2 * c], W[2 * c + 1]], axis=1)
            return np.ascontiguousarray(
                (SW * wc).reshape(EO_, 128, 128).transpose(1, 0, 2)).astype(FP8)
        m["wq"] = tile_w(Wq)
        m["wk"] = tile_w(Wk)
        m["wv"] = tile_w(Wv)
        rows = np.concatenate([
            xflat[HSL * c:HSL * (c + 1)],
            xflat[T + HSL * c:T + HSL * (c + 1)]], axis=0)
        m["x_slice"] = np.ascontiguousarray(
            rows.reshape(TSL // 128, 128, E).transpose(1, 0, 2))
        in_maps.append(m)
    return in_maps


def _enable_trace_hook():
    """Register the axon NTFF profile hook (synthesize antenv.axon_hooks)."""
    import types
    import antenv  # noqa: F401

    if "antenv.axon_hooks" not in sys.modules:
        mod = types.ModuleType("antenv.axon_hooks")
        mod._hook = None
        mod.set_axon_ntff_profile_hook = lambda h: setattr(mod, "_hook", h)
        mod.get_axon_ntff_profile_hook = lambda: mod._hook
        sys.modules["antenv.axon_hooks"] = mod
        antenv.axon_hooks = mod
    mod = sys.modules["antenv.axon_hooks"]
    if mod.get_axon_ntff_profile_hook() is None:
        if "/root/.axon_site" not in sys.path:
            sys.path.insert(0, "/root/.axon_site")
        from trn_agent_boot.trn_boot import _ntff_profile_via_ctypes
        mod.set_axon_ntff_profile_hook(
            _ntff_profile_via_ctypes("/opt/axon/libaxon_pjrt.so"))


def run(inputs, trace=False):
    """Returns (full_output [B,T,E] f32, BassKernelResults)."""
    from concourse import bass_utils

    if "nc" not in _cache:
        _cache["nc"] = _build()
    nc = _cache["nc"]
    in_maps = _make_in_maps(inputs)
    if trace:
        _enable_trace_hook()
    res = bass_utils.run_bass_kernel_spmd(
        nc, in_maps, core_ids=list(range(N_CORES)), trace=trace)
    full = np.empty((NTOK, E), dtype=np.float32)
    for c in range(N_CORES):
        o = res.results[c]["out"]
        full[HSL * c:HSL * (c + 1)] = o[:HSL]
        full[T + HSL * c:T + HSL * (c + 1)] = o[HSL:]
    return full.reshape(B, T, E), res


def kernel(**inputs):
    out, _ = run(inputs, trace=False)
    return out
